# revision 21
# baseline (speedup 1.0000x reference)
"""BertSelfAttention Trainium2 kernel.

Shapes: hidden_states [S=1024, B=4, D=1024], H=16 heads of DH=64.
Sharding: 2 heads per core (8 cores). Each core receives the full hidden
states (pre-transposed + bf16-cast on host) and a 128-row slice of each
projection weight, computes the full attention chain for its two heads with
no cross-core communication, and writes ctx^T per (batch, head).

Device-side layout tricks:
  - scores are computed transposed (scoresT[u, t] = q_t . k_u) so the
    additive attention mask (per key position u) is a per-partition bias
    that fuses into the Exp activation: probsT = exp(scores/8 + mask).
  - V carries a prepended ones-column, so the AV matmul produces the
    softmax denominator in row 0 of ctxT for free; normalization is
    reciprocal_approx_fast + gpsimd partition_broadcast + one multiply.
  - work is organized as 8 passes, one per (batch, head): scores/exp
    stream per 128-key tile, and the AV accumulation chases the exp
    output 2 key-tiles behind WITHIN the pass, so there is no epilogue
    AV block at the end of the kernel.  V projection of batch bi and
    Q/K projection of batch bi+1 are woven into the pass as PE filler,
    in DMA arrival order.

Measured on trn2 (8 cores): see test.py; rel err ~4e-3 vs the fp32
reference (bf16 matmul inputs; fp32 accumulation throughout).
"""

import os
import numpy as np
import ml_dtypes

S, B, D, H = 1024, 4, 1024, 16
DH = D // H          # 64
NCORES = 8
HPC = H // NCORES    # heads per core = 2
P = 128              # partitions / d-tile / u-tile
DCH = D // P         # 8 contraction tiles
BS = B * S           # 4096 flattened (b, s)
CH = 512             # matmul free-dim chunk (fp32 psum bank limit)
LAG = 2              # AV runs this many u-tiles behind scores/exp

_compiled = {}
last_exec_time_ns = None
last_results = None


def _build(NT):
    import concourse.bacc as bacc
    import concourse.mybir as mybir
    import concourse.tile as tile
    from contextlib import ExitStack

    f32 = mybir.dt.float32
    bf16 = mybir.dt.bfloat16
    AF = mybir.ActivationFunctionType

    nc = bacc.Bacc("TRN2", target_bir_lowering=False, debug=False,
                   num_devices=NCORES)

    hT_d = nc.dram_tensor("hT", [D, BS], bf16, kind="ExternalInput")
    # weights pre-tiled on host to [p, dc*m] so the DMA moves one
    # contiguous 2KB line per partition (256B strided lines run ~6x slower)
    wqT_d = nc.dram_tensor("wqT", [P, DCH * P], bf16, kind="ExternalInput")
    wkT_d = nc.dram_tensor("wkT", [P, DCH * P], bf16, kind="ExternalInput")
    wvT_d = nc.dram_tensor("wvT", [P, DCH * P], bf16, kind="ExternalInput")
    # packed per-partition constants: [bq | bk | bvb(128) | maskT(8*4)]
    misc_d = nc.dram_tensor("misc", [P, 2 + P + DCH * B], f32,
                            kind="ExternalInput")
    out_d = nc.dram_tensor("out", [B, HPC, DH, S], f32, kind="ExternalOutput")

    with tile.TileContext(nc) as tc, ExitStack() as ctx:
        persist = ctx.enter_context(tc.tile_pool(name="persist", bufs=1))
        probs_pool = ctx.enter_context(tc.tile_pool(name="probs", bufs=8))
        small = ctx.enter_context(tc.tile_pool(name="small", bufs=4))
        out_pool = ctx.enter_context(tc.tile_pool(name="outp", bufs=4))
        ps_mm = ctx.enter_context(tc.tile_pool(name="ps_mm", bufs=2, space="PSUM"))
        ps_sc = ctx.enter_context(tc.tile_pool(name="ps_sc", bufs=2, space="PSUM"))
        ps_ctx = ctx.enter_context(tc.tile_pool(name="ps_ctx", bufs=2, space="PSUM"))

        # ---- persistent SBUF tensors ----
        hT_sb = persist.tile([P, DCH, BS], bf16)        # hidden^T, d-tiled
        wq_sb = persist.tile([P, DCH, P], bf16)
        wk_sb = persist.tile([P, DCH, P], bf16)
        wv_sb = persist.tile([P, DCH, P], bf16)
        misc_sb = persist.tile([P, 2 + P + DCH * B], f32)
        qT_sb = persist.tile([P, BS], bf16)             # Q^T [i, t]
        kT_sb = persist.tile([P, BS], bf16)             # K^T [i, t]
        # V in [t, j] layout + ones column per head: [t-part, t-tile, head, DH+1]
        v_sb = persist.tile([P, BS // P, HPC, DH + 1], bf16)
        dummy_sb = persist.tile([P, CH], bf16)

        bq_sb = misc_sb[:, 0:1]
        bk_sb = misc_sb[:, 1:2]
        bvb_sb = misc_sb[:, 2:2 + P]

        def mask_bias(uc, bi):
            c = 2 + P + uc * B + bi
            return misc_sb[:, c:c + 1]

        # ---- HAM warmup: dead matmuls keep the PE busy while the first
        # weight/hidden DMAs land.  All into ONE psum tile (pure in-order
        # WAW on the PE, no cross-engine sems) so they run back-to-back and
        # fill the HAM activity window — the clock boosts ~3.4us in instead
        # of ~7us into the real work.
        nc.vector.memset(dummy_sb[:], 0.0)
        # prefetch the ACT exp table (~2.7us ACT_TABLE_LOAD) long before the
        # first real exp, so it never blocks the scores->exp->AV chain
        warm_act = small.tile([1, 8], f32, name="warm_act")
        nc.scalar.activation(warm_act[:], dummy_sb[0:1, 0:8], AF.Exp)
        d_ps = ps_sc.tile([P, CH], f32, tag="sc", name="d_ps")
        for _ in range(4):
            nc.tensor.matmul(d_ps[:], dummy_sb[:, 0:P], dummy_sb[:],
                             start=True, stop=True)

        # ---- input DMAs ----
        # Ordered for time-to-first-score-matmul: wq/wk, then batch 0's hT
        # pieces (dc-minor) spread over FOUR HWDGE queues (SP/ACT/DVE/POOL)
        # so the ~2.3MB the prologue needs lands in ~3us instead of ~12,
        # then wv/misc, then the remaining batches on the idle-ish queues.
        hT_re = hT_d.ap().rearrange("(dc p) t -> p dc t", p=P)

        def hT_piece(q, dc, eng):
            qsl = slice(q * S, (q + 1) * S)
            eng.dma_start(hT_sb[:, dc, qsl], hT_re[:, dc, qsl])

        nc.sync.dma_start(wq_sb[:], wqT_d.ap().rearrange("p (dc m) -> p dc m", m=P))
        nc.scalar.dma_start(wk_sb[:], wkT_d.ap().rearrange("p (dc m) -> p dc m", m=P))
        b0_engs = (nc.sync, nc.scalar, nc.gpsimd)
        for dc in range(DCH):
            hT_piece(0, dc, b0_engs[dc % 3])
        nc.sync.dma_start(misc_sb[:], misc_d.ap())
        nc.sync.dma_start(wv_sb[:], wvT_d.ap().rearrange("p (dc m) -> p dc m", m=P))
        for q in range(1, B):
            for dc in range(DCH):
                hT_piece(q, dc, nc.sync if dc % 2 == 0 else nc.gpsimd)

        nc.vector.memset(v_sb[:, :, :, 0:1], 1.0)

        scale = 1.0 / float(np.sqrt(DH))

        # ---- projection thunks (PE filler woven into the passes) ----
        # Q covers all S queries per batch; K/V only the first NT[bi]*128
        # packed (unmasked-first) key positions.
        def emit_qk_chunk(w_sb, b_sb, dst, bi, ci, width):
            sl = slice(bi * S + ci * CH, bi * S + ci * CH + width)
            qk_ps = ps_mm.tile([P, CH], f32, tag="mm", name="qk_ps")
            for dc in range(DCH):
                nc.tensor.matmul(
                    qk_ps[:, 0:width], w_sb[:, dc, :], hT_sb[:, dc, sl],
                    start=(dc == 0), stop=(dc == DCH - 1))
            nc.vector.tensor_scalar_add(dst[:, sl], qk_ps[:, 0:width], b_sb[:])

        def emit_v_tile(tt):
            tsl = slice(tt * P, (tt + 1) * P)
            v_ps = ps_mm.tile([P, P], f32, tag="mm", name="v_ps")
            for dc in range(DCH):
                nc.tensor.matmul(
                    v_ps[:], hT_sb[:, dc, tsl], wv_sb[:, dc, :],
                    start=(dc == 0), stop=(dc == DCH - 1))
            nc.vector.tensor_add(
                v_sb[:, tt, 0:HPC, 1:DH + 1],
                v_ps[:].rearrange("p (h j) -> p h j", j=DH),
                bvb_sb[:].rearrange("p (h j) -> p h j", j=DH))

        def k_widths(bi):
            kcols = NT[bi] * P
            ws = []
            c = 0
            while kcols > 0:
                ws.append((c, min(CH, kcols)))
                kcols -= CH
                c += 1
            return ws

        def qk_thunks(bi):
            th = [lambda c=ci: emit_qk_chunk(wq_sb, bq_sb, qT_sb, bi, c, CH)
                  for ci in range(2)]
            th += [lambda c=ci, w=wd: emit_qk_chunk(wk_sb, bk_sb, kT_sb, bi, c, w)
                   for ci, wd in k_widths(bi)]
            return th

        def v_thunks(bi):
            return [lambda t=tt: emit_v_tile(t)
                    for tt in range(8 * bi, 8 * bi + NT[bi])]

        # ---- prologue: batch 0's Q (both chunks) + K chunk 0, dc-major so
        # all three PSUM accumulation groups chase the arriving hT pieces
        # concurrently.  K chunk 1 is the first filler of pass (0,h0): the
        # first 4 u-tiles of scores only need K chunk 0, so scores start
        # ~1.7us sooner.
        kw0 = min(CH, NT[0] * P)
        pro_pools = (ps_mm, ps_mm, ps_sc)
        pro_specs = [(wq_sb, bq_sb, qT_sb, 0, CH), (wq_sb, bq_sb, qT_sb, 1, CH),
                     (wk_sb, bk_sb, kT_sb, 0, kw0)]
        pro_ps = [pool.tile([P, CH], f32, tag="mm" if pool is ps_mm else "sc",
                            name="pro_ps") for pool in pro_pools]
        for dc in range(DCH):
            for g, (w_sb, b_sb, dst, ci, wd) in enumerate(pro_specs):
                nc.tensor.matmul(
                    pro_ps[g][:, 0:wd], w_sb[:, dc, :],
                    hT_sb[:, dc, ci * CH:ci * CH + wd],
                    start=(dc == 0), stop=(dc == DCH - 1))
        for g, (w_sb, b_sb, dst, ci, wd) in enumerate(pro_specs):
            osl = slice(ci * CH, ci * CH + wd)
            if g < 2:      # split the bias-adds across DVE and ACT
                nc.vector.tensor_scalar_add(dst[:, osl], pro_ps[g][:, 0:wd],
                                            b_sb[:])
            else:
                nc.scalar.activation(dst[:, osl], pro_ps[g][:, 0:wd],
                                     AF.Identity, bias=b_sb[:])

        # ---- per-(batch, head) passes ----
        # Per pass: 8 u-tiles of scoresT -> exp (ACT-paced via the 2-slot
        # sc psum rotation), AV chasing LAG u-tiles behind, projection
        # filler popped once per step.  Norm + output DMA at pass end
        # overlap the next pass.
        def emit_av(bi, hl, ctx_tiles, pps, uc):
            # mid-pass AV step; the final u-tile group is drained c2-major
            # at pass end, so stop is never set here
            for c2 in range(2):
                nc.tensor.matmul(
                    ctx_tiles[c2][:],
                    v_sb[:, bi * 8 + uc, hl, :],
                    pps[uc][:, c2 * CH:(c2 + 1) * CH],
                    start=(uc == 0), stop=False)

        def emit_norm(bi, hl, ctx_tiles):
            # ctx row 0 = denominator, rows 1..DH = unnormalized ctx^T.
            o_sb = out_pool.tile([DH + 1, S], f32, name="o_sb")
            for c2 in range(2):
                csl = slice(c2 * CH, (c2 + 1) * CH)
                ctx_ps = ctx_tiles[c2]
                rcp_sb = small.tile([1, CH], f32, name="rcp_sb")
                nc.vector.reciprocal_approx_fast(rcp_sb[:], ctx_ps[0:1, :])
                rcpb_sb = small.tile([DH + 1, CH], f32, name="rcpb_sb")
                nc.gpsimd.partition_broadcast(rcpb_sb[:], rcp_sb[:])
                nc.vector.tensor_mul(o_sb[:, csl], ctx_ps[:], rcpb_sb[:])
                nc.sync.dma_start(out_d.ap()[bi, hl, :, csl],
                                  o_sb[1:DH + 1, csl])

        # filler queues per pass.  Each projection finishes at least one
        # full pass before its consumer pass, so the consumer's matmuls
        # carry no cross-engine waits (the PE's vector clock has already
        # observed the producing DVE/ACT ticks).  v(b0) is the exception:
        # it chases pass (0,h0)'s own AV, one u-tile pair ahead — hence the
        # front-loaded pop distribution below.
        k1_thunk = [lambda c=ci, w=wd: emit_qk_chunk(wk_sb, bk_sb, kT_sb,
                                                     0, c, w)
                    for ci, wd in k_widths(0)[1:]]
        qk1 = qk_thunks(1)
        fillers = {
            (0, 0): k1_thunk + v_thunks(0) + qk1[:2],
            (0, 1): qk1[2:] + v_thunks(1),
            (1, 0): qk_thunks(2),
            (1, 1): v_thunks(2),
            (2, 0): qk_thunks(3),
            (2, 1): v_thunks(3),
            (3, 0): [], (3, 1): [],
        }

        def pop_split(n, g):
            base, extra = divmod(n, g)
            return [base + (1 if i < extra else 0) for i in range(g)]

        # u-tiles processed in pairs: each group emits scores+exp for two
        # u-tiles back-to-back (both sc psum slots in flight), then the AV
        # pair one group behind, then the group's filler thunks.  Coarser
        # grouping halves the number of sem-guarded PE stationary switches.
        for bi in range(B):
            nt = NT[bi]
            groups = [tuple(range(g, min(g + 2, nt))) for g in range(0, nt, 2)]
            for hl in range(HPC):
                queue = fillers[(bi, hl)]
                pops = pop_split(len(queue), len(groups))
                hsl = slice(hl * DH, (hl + 1) * DH)
                ctx_tiles = [ps_ctx.tile([DH + 1, CH], f32, tag="ctx",
                                         name="ctx_ps") for _ in range(2)]
                pps = []
                for gi, grp in enumerate(groups):
                    for uc in grp:
                        usl = slice(bi * S + uc * P, bi * S + (uc + 1) * P)
                        sc = ps_sc.tile([P, S], f32, tag="sc", name="sc_ps")
                        for c2 in range(2):
                            qsl = slice(bi * S + c2 * CH,
                                        bi * S + (c2 + 1) * CH)
                            nc.tensor.matmul(
                                sc[:, c2 * CH:(c2 + 1) * CH],
                                kT_sb[hsl, usl], qT_sb[hsl, qsl],
                                start=True, stop=True)
                        pp = probs_pool.tile([P, S], bf16, name="pp")
                        nc.scalar.activation(pp[:], sc[:], AF.Exp,
                                             bias=mask_bias(uc, bi),
                                             scale=scale)
                        pps.append(pp)
                    if gi >= 1:
                        for uc in groups[gi - 1]:
                            emit_av(bi, hl, ctx_tiles, pps, uc)
                    for _ in range(pops[gi]):
                        if queue:
                            queue.pop(0)()
                # drain the last AV group c2-major: chunk 0's accumulation
                # stops early, so its norm chain (rcp -> broadcast -> mul ->
                # DMA) overlaps chunk 1's AV streams.
                for c2 in range(2):
                    for uc in groups[-1]:
                        nc.tensor.matmul(
                            ctx_tiles[c2][:],
                            v_sb[:, bi * 8 + uc, hl, :],
                            pps[uc][:, c2 * CH:(c2 + 1) * CH],
                            start=False, stop=(uc == nt - 1))
                    if queue:
                        queue.pop(0)()
                emit_norm(bi, hl, ctx_tiles)

    nc.compile()
    return nc


def _get_nc(NT):
    nc = _compiled.get(NT)
    if nc is None:
        nc = _compiled[NT] = _build(NT)
    return nc


def prepare_in_maps(hidden_states, attention_mask, Wq, bq, Wk, bk, Wv, bv):
    """Returns (in_maps, perms, NT).

    Tokens of each batch are permuted so masked key positions (additive
    mask <= -5000 => exp underflows to exactly 0) come last; the kernel
    then only computes scores/AV for the first NT[bi] 128-key tiles.
    Queries are permuted identically (the query axis is pointwise through
    the whole attention chain) and un-permuted on the host when gathering
    the output, so no second hidden-states copy is needed.
    """
    bf16 = ml_dtypes.bfloat16

    hs = np.asarray(hidden_states, dtype=np.float32)            # [S, B, D]
    mask2 = np.asarray(attention_mask, dtype=np.float32).reshape(B, S)
    perms, NT = [], []
    hsp = np.empty_like(hs)
    maskp = np.empty_like(mask2)
    for bi in range(B):
        masked = mask2[bi] <= -5000.0
        order = np.argsort(masked, kind="stable")   # unmasked keys first
        nk = int((~masked).sum())
        NT.append(min(DCH, max(1, -(-nk // P))))
        perms.append(order)
        hsp[:, bi, :] = hs[order, bi, :]
        maskp[bi] = mask2[bi, order]
    NT = tuple(NT)

    hT = np.ascontiguousarray(hsp.transpose(2, 1, 0).reshape(D, BS)).astype(bf16)
    maskT = np.ascontiguousarray(maskp.T)                       # [S, B]
    Wq = np.asarray(Wq, dtype=np.float32)
    Wk = np.asarray(Wk, dtype=np.float32)
    Wv = np.asarray(Wv, dtype=np.float32)
    bq = np.asarray(bq, dtype=np.float32)
    bk = np.asarray(bk, dtype=np.float32)
    bv = np.asarray(bv, dtype=np.float32)

    # maskT packed as [p, uc, b] -> [128, 32]
    mask_pk = maskT.reshape(DCH, P, B).transpose(1, 0, 2).reshape(P, DCH * B)
    in_maps = []
    for c in range(NCORES):
        sl = slice(P * c, P * (c + 1))
        misc = np.empty((P, 2 + P + DCH * B), dtype=np.float32)
        misc[:, 0] = bq[sl]
        misc[:, 1] = bk[sl]
        misc[:, 2:2 + P] = np.broadcast_to(bv[sl][None, :], (P, P))
        misc[:, 2 + P:] = mask_pk
        def pack_w(W):
            # [D, 128] -> [p, dc*m]: per-partition 2KB contiguous DMA lines
            wt = np.ascontiguousarray(W[sl, :].T).astype(bf16)
            return np.ascontiguousarray(
                wt.reshape(DCH, P, P).transpose(1, 0, 2).reshape(P, DCH * P))

        in_maps.append({
            "hT": hT,
            "wqT": pack_w(Wq),
            "wkT": pack_w(Wk),
            "wvT": pack_w(Wv),
            "misc": misc,
        })
    return in_maps, perms, NT


def kernel(hidden_states, attention_mask, Wq, bq, Wk, bk, Wv, bv):
    global last_exec_time_ns, last_results
    from concourse.bass_utils import run_bass_kernel_spmd

    in_maps, perms, NT = prepare_in_maps(hidden_states, attention_mask,
                                         Wq, bq, Wk, bk, Wv, bv)
    nc = _get_nc(NT)

    trace = bool(int(os.environ.get("KERNEL_TRACE", "0")))
    tmpdir = os.environ.get("KERNEL_TRACE_DIR") or None
    res = run_bass_kernel_spmd(nc, in_maps, core_ids=list(range(NCORES)),
                               trace=trace, tmpdir=tmpdir)
    last_exec_time_ns = res.exec_time_ns
    last_results = res

    # gather: per-core out [B, HPC, DH, S] -> full [S, B, D], un-permuting
    # the query axis per batch
    outs = np.stack([np.asarray(res.results[c]["out"]) for c in range(NCORES)],
                    axis=0)                                     # [C, B, HPC, DH, S]
    full_p = outs.transpose(4, 1, 0, 2, 3).reshape(S, B, D)     # t', b, (c, hl, j)
    full = np.empty_like(full_p)
    for bi in range(B):
        full[perms[bi], bi, :] = full_p[:, bi, :]
    return np.ascontiguousarray(full.astype(np.float32))


# revision 23
# speedup vs baseline: 1.0112x; 1.0112x over previous
"""BertSelfAttention Trainium2 kernel.

Shapes: hidden_states [S=1024, B=4, D=1024], H=16 heads of DH=64.
Sharding: 2 heads per core (8 cores). Each core receives the full hidden
states (pre-transposed + bf16-cast on host) and a 128-row slice of each
projection weight, computes the full attention chain for its two heads with
no cross-core communication, and writes ctx^T per (batch, head).

Device-side layout tricks:
  - masked-key packing: tokens of each batch are permuted on the host so
    masked key positions (additive mask -10000 => exp underflows to
    exactly 0.0) come last; the kernel only computes scores/exp/AV for
    the first NT[bi] (typically 7 of 8) 128-key tiles.  Queries are
    permuted identically (the query axis is pointwise through the chain)
    and un-permuted in the host-side output gather, so the SAME hidden
    buffer feeds Q, K and V and the result is bit-identical.
  - scores are computed transposed (scoresT[u, t] = q_t . k_u) so the
    additive attention mask (per key position u) is a per-partition bias
    that fuses into the Exp activation: probsT = exp(scores/8 + mask).
  - V carries a prepended ones-column, so the AV matmul produces the
    softmax denominator in row 0 of ctxT for free; normalization is
    reciprocal_approx_fast + gpsimd partition_broadcast + one multiply.
  - work is organized as 8 passes, one per (batch, head): scores/exp
    stream per pair of 128-key tiles (pairing halves the sem-guarded PE
    stationary switches), and the AV accumulation chases the exp output
    one pair behind WITHIN the pass, so there is no epilogue AV block.
    V and Q/K projections are woven into the passes as PE filler at
    least one pass ahead of their consumer, in DMA arrival order.
  - weights are pre-tiled on the host so every input DMA moves 2KB
    contiguous per-partition lines; batch 0's hidden pieces split across
    both HWDGE queues; dense single-tile warmup matmuls fill the HAM
    activity window so the 2.4GHz boost arrives during the prologue.

Measured on trn2 (8 cores): see test.py; rel err ~4.4e-3 vs the fp32
reference (bf16 matmul inputs; fp32 accumulation throughout).
"""

import os
import numpy as np
import ml_dtypes

S, B, D, H = 1024, 4, 1024, 16
DH = D // H          # 64
NCORES = 8
HPC = H // NCORES    # heads per core = 2
P = 128              # partitions / d-tile / u-tile
DCH = D // P         # 8 contraction tiles
BS = B * S           # 4096 flattened (b, s)
CH = 512             # matmul free-dim chunk (fp32 psum bank limit)
LAG = 2              # AV runs this many u-tiles behind scores/exp

_compiled = {}
last_exec_time_ns = None
last_results = None


def _build(NT):
    import concourse.bacc as bacc
    import concourse.mybir as mybir
    import concourse.tile as tile
    from contextlib import ExitStack

    f32 = mybir.dt.float32
    bf16 = mybir.dt.bfloat16
    AF = mybir.ActivationFunctionType

    nc = bacc.Bacc("TRN2", target_bir_lowering=False, debug=False,
                   num_devices=NCORES)

    hT_d = nc.dram_tensor("hT", [D, BS], bf16, kind="ExternalInput")
    # weights pre-tiled on host to [p, dc*m] so the DMA moves one
    # contiguous 2KB line per partition (256B strided lines run ~6x slower)
    wqT_d = nc.dram_tensor("wqT", [P, DCH * P], bf16, kind="ExternalInput")
    wkT_d = nc.dram_tensor("wkT", [P, DCH * P], bf16, kind="ExternalInput")
    wvT_d = nc.dram_tensor("wvT", [P, DCH * P], bf16, kind="ExternalInput")
    # packed per-partition constants: [bq | bk | bvb(128) | maskT(8*4)]
    misc_d = nc.dram_tensor("misc", [P, 2 + P + DCH * B], f32,
                            kind="ExternalInput")
    out_d = nc.dram_tensor("out", [B, HPC, DH, S], f32, kind="ExternalOutput")

    with tile.TileContext(nc) as tc, ExitStack() as ctx:
        persist = ctx.enter_context(tc.tile_pool(name="persist", bufs=1))
        probs_pool = ctx.enter_context(tc.tile_pool(name="probs", bufs=8))
        small = ctx.enter_context(tc.tile_pool(name="small", bufs=4))
        out_pool = ctx.enter_context(tc.tile_pool(name="outp", bufs=4))
        ps_mm = ctx.enter_context(tc.tile_pool(name="ps_mm", bufs=2, space="PSUM"))
        ps_sc = ctx.enter_context(tc.tile_pool(name="ps_sc", bufs=2, space="PSUM"))
        ps_ctx = ctx.enter_context(tc.tile_pool(name="ps_ctx", bufs=2, space="PSUM"))

        # ---- persistent SBUF tensors ----
        hT_sb = persist.tile([P, DCH, BS], bf16)        # hidden^T, d-tiled
        wq_sb = persist.tile([P, DCH, P], bf16)
        wk_sb = persist.tile([P, DCH, P], bf16)
        wv_sb = persist.tile([P, DCH, P], bf16)
        misc_sb = persist.tile([P, 2 + P + DCH * B], f32)
        qT_sb = persist.tile([P, BS], bf16)             # Q^T [i, t]
        kT_sb = persist.tile([P, BS], bf16)             # K^T [i, t]
        # V in [t, j] layout + ones column per head: [t-part, t-tile, head, DH+1]
        v_sb = persist.tile([P, BS // P, HPC, DH + 1], bf16)
        dummy_sb = persist.tile([P, CH], bf16)

        bq_sb = misc_sb[:, 0:1]
        bk_sb = misc_sb[:, 1:2]
        bvb_sb = misc_sb[:, 2:2 + P]

        def mask_bias(uc, bi):
            c = 2 + P + uc * B + bi
            return misc_sb[:, c:c + 1]

        # ---- HAM warmup: dead matmuls keep the PE busy while the first
        # weight/hidden DMAs land.  All into ONE psum tile (pure in-order
        # WAW on the PE, no cross-engine sems) so they run back-to-back and
        # fill the HAM activity window — the clock boosts ~3.4us in instead
        # of ~7us into the real work.
        nc.vector.memset(dummy_sb[:], 0.0)
        # prefetch the ACT exp table (~2.7us ACT_TABLE_LOAD) long before the
        # first real exp, so it never blocks the scores->exp->AV chain
        warm_act = small.tile([1, 8], f32, name="warm_act")
        nc.scalar.activation(warm_act[:], dummy_sb[0:1, 0:8], AF.Exp)
        d_ps = ps_sc.tile([P, CH], f32, tag="sc", name="d_ps")
        for _ in range(4):
            nc.tensor.matmul(d_ps[:], dummy_sb[:, 0:P], dummy_sb[:],
                             start=True, stop=True)

        # ---- input DMAs ----
        # Ordered for time-to-first-score-matmul: wq/wk, then batch 0's hT
        # pieces (dc-minor) spread over FOUR HWDGE queues (SP/ACT/DVE/POOL)
        # so the ~2.3MB the prologue needs lands in ~3us instead of ~12,
        # then wv/misc, then the remaining batches on the idle-ish queues.
        hT_re = hT_d.ap().rearrange("(dc p) t -> p dc t", p=P)

        def hT_piece(q, dc, eng):
            qsl = slice(q * S, (q + 1) * S)
            eng.dma_start(hT_sb[:, dc, qsl], hT_re[:, dc, qsl])

        nc.sync.dma_start(wq_sb[:], wqT_d.ap().rearrange("p (dc m) -> p dc m", m=P))
        nc.scalar.dma_start(wk_sb[:], wkT_d.ap().rearrange("p (dc m) -> p dc m", m=P))
        for dc in range(DCH):
            hT_piece(0, dc, nc.sync if dc % 2 == 0 else nc.scalar)
        nc.sync.dma_start(misc_sb[:], misc_d.ap())
        nc.sync.dma_start(wv_sb[:], wvT_d.ap().rearrange("p (dc m) -> p dc m", m=P))
        for q in range(1, B):
            for dc in range(DCH):
                hT_piece(q, dc, nc.sync)

        nc.vector.memset(v_sb[:, :, :, 0:1], 1.0)

        scale = 1.0 / float(np.sqrt(DH))

        # ---- projection thunks (PE filler woven into the passes) ----
        # Q covers all S queries per batch; K/V only the first NT[bi]*128
        # packed (unmasked-first) key positions.
        def emit_qk_chunk(w_sb, b_sb, dst, bi, ci, width):
            sl = slice(bi * S + ci * CH, bi * S + ci * CH + width)
            qk_ps = ps_mm.tile([P, CH], f32, tag="mm", name="qk_ps")
            for dc in range(DCH):
                nc.tensor.matmul(
                    qk_ps[:, 0:width], w_sb[:, dc, :], hT_sb[:, dc, sl],
                    start=(dc == 0), stop=(dc == DCH - 1))
            nc.vector.tensor_scalar_add(dst[:, sl], qk_ps[:, 0:width], b_sb[:])

        def emit_v_tile(tt):
            tsl = slice(tt * P, (tt + 1) * P)
            v_ps = ps_mm.tile([P, P], f32, tag="mm", name="v_ps")
            for dc in range(DCH):
                nc.tensor.matmul(
                    v_ps[:], hT_sb[:, dc, tsl], wv_sb[:, dc, :],
                    start=(dc == 0), stop=(dc == DCH - 1))
            nc.vector.tensor_add(
                v_sb[:, tt, 0:HPC, 1:DH + 1],
                v_ps[:].rearrange("p (h j) -> p h j", j=DH),
                bvb_sb[:].rearrange("p (h j) -> p h j", j=DH))

        def k_widths(bi):
            kcols = NT[bi] * P
            ws = []
            c = 0
            while kcols > 0:
                ws.append((c, min(CH, kcols)))
                kcols -= CH
                c += 1
            return ws

        def qk_thunks(bi):
            th = [lambda c=ci: emit_qk_chunk(wq_sb, bq_sb, qT_sb, bi, c, CH)
                  for ci in range(2)]
            th += [lambda c=ci, w=wd: emit_qk_chunk(wk_sb, bk_sb, kT_sb, bi, c, w)
                   for ci, wd in k_widths(bi)]
            return th

        def v_thunks(bi):
            return [lambda t=tt: emit_v_tile(t)
                    for tt in range(8 * bi, 8 * bi + NT[bi])]

        # ---- prologue: batch 0's Q (both chunks) + K chunk 0, dc-major so
        # all three PSUM accumulation groups chase the arriving hT pieces
        # concurrently.  K chunk 1 is the first filler of pass (0,h0): the
        # first 4 u-tiles of scores only need K chunk 0, so scores start
        # ~1.7us sooner.
        kw0 = min(CH, NT[0] * P)
        pro_pools = (ps_mm, ps_mm, ps_sc)
        pro_specs = [(wq_sb, bq_sb, qT_sb, 0, CH), (wq_sb, bq_sb, qT_sb, 1, CH),
                     (wk_sb, bk_sb, kT_sb, 0, kw0)]
        pro_ps = [pool.tile([P, CH], f32, tag="mm" if pool is ps_mm else "sc",
                            name="pro_ps") for pool in pro_pools]
        for dc in range(DCH):
            for g, (w_sb, b_sb, dst, ci, wd) in enumerate(pro_specs):
                nc.tensor.matmul(
                    pro_ps[g][:, 0:wd], w_sb[:, dc, :],
                    hT_sb[:, dc, ci * CH:ci * CH + wd],
                    start=(dc == 0), stop=(dc == DCH - 1))
        for g, (w_sb, b_sb, dst, ci, wd) in enumerate(pro_specs):
            osl = slice(ci * CH, ci * CH + wd)
            if g < 2:      # split the bias-adds across DVE and ACT
                nc.vector.tensor_scalar_add(dst[:, osl], pro_ps[g][:, 0:wd],
                                            b_sb[:])
            else:
                nc.scalar.activation(dst[:, osl], pro_ps[g][:, 0:wd],
                                     AF.Identity, bias=b_sb[:])

        # ---- per-(batch, head) passes ----
        # Per pass: 8 u-tiles of scoresT -> exp (ACT-paced via the 2-slot
        # sc psum rotation), AV chasing LAG u-tiles behind, projection
        # filler popped once per step.  Norm + output DMA at pass end
        # overlap the next pass.
        def emit_av(bi, hl, ctx_tiles, pps, uc):
            # mid-pass AV step; the final u-tile group is drained c2-major
            # at pass end, so stop is never set here
            for c2 in range(2):
                nc.tensor.matmul(
                    ctx_tiles[c2][:],
                    v_sb[:, bi * 8 + uc, hl, :],
                    pps[uc][:, c2 * CH:(c2 + 1) * CH],
                    start=(uc == 0), stop=False)

        def emit_norm(bi, hl, ctx_tiles):
            # ctx row 0 = denominator, rows 1..DH = unnormalized ctx^T.
            o_sb = out_pool.tile([DH + 1, S], f32, name="o_sb")
            for c2 in range(2):
                csl = slice(c2 * CH, (c2 + 1) * CH)
                ctx_ps = ctx_tiles[c2]
                rcp_sb = small.tile([1, CH], f32, name="rcp_sb")
                nc.vector.reciprocal_approx_fast(rcp_sb[:], ctx_ps[0:1, :])
                rcpb_sb = small.tile([DH + 1, CH], f32, name="rcpb_sb")
                nc.gpsimd.partition_broadcast(rcpb_sb[:], rcp_sb[:])
                nc.vector.tensor_mul(o_sb[:, csl], ctx_ps[:], rcpb_sb[:])
                nc.sync.dma_start(out_d.ap()[bi, hl, :, csl],
                                  o_sb[1:DH + 1, csl])

        # filler queues per pass.  Each projection finishes at least one
        # full pass before its consumer pass, so the consumer's matmuls
        # carry no cross-engine waits (the PE's vector clock has already
        # observed the producing DVE/ACT ticks).  v(b0) is the exception:
        # it chases pass (0,h0)'s own AV, one u-tile pair ahead — hence the
        # front-loaded pop distribution below.
        k1_thunk = [lambda c=ci, w=wd: emit_qk_chunk(wk_sb, bk_sb, kT_sb,
                                                     0, c, w)
                    for ci, wd in k_widths(0)[1:]]
        qk1 = qk_thunks(1)
        fillers = {
            (0, 0): k1_thunk + v_thunks(0) + qk1[:2],
            (0, 1): qk1[2:] + v_thunks(1),
            (1, 0): qk_thunks(2),
            (1, 1): v_thunks(2),
            (2, 0): qk_thunks(3),
            (2, 1): v_thunks(3),
            (3, 0): [], (3, 1): [],
        }

        def pop_split(n, g):
            base, extra = divmod(n, g)
            return [base + (1 if i < extra else 0) for i in range(g)]

        # u-tiles processed in pairs: each group emits scores+exp for two
        # u-tiles back-to-back (both sc psum slots in flight), then the AV
        # pair one group behind, then the group's filler thunks.  Coarser
        # grouping halves the number of sem-guarded PE stationary switches.
        for bi in range(B):
            nt = NT[bi]
            groups = [tuple(range(g, min(g + 2, nt))) for g in range(0, nt, 2)]
            for hl in range(HPC):
                queue = fillers[(bi, hl)]
                pops = pop_split(len(queue), len(groups))
                hsl = slice(hl * DH, (hl + 1) * DH)
                ctx_tiles = [ps_ctx.tile([DH + 1, CH], f32, tag="ctx",
                                         name="ctx_ps") for _ in range(2)]
                pps = []
                for gi, grp in enumerate(groups):
                    for uc in grp:
                        usl = slice(bi * S + uc * P, bi * S + (uc + 1) * P)
                        sc = ps_sc.tile([P, S], f32, tag="sc", name="sc_ps")
                        for c2 in range(2):
                            qsl = slice(bi * S + c2 * CH,
                                        bi * S + (c2 + 1) * CH)
                            nc.tensor.matmul(
                                sc[:, c2 * CH:(c2 + 1) * CH],
                                kT_sb[hsl, usl], qT_sb[hsl, qsl],
                                start=True, stop=True)
                        pp = probs_pool.tile([P, S], bf16, name="pp")
                        nc.scalar.activation(pp[:], sc[:], AF.Exp,
                                             bias=mask_bias(uc, bi),
                                             scale=scale)
                        pps.append(pp)
                    if gi >= 1:
                        for uc in groups[gi - 1]:
                            emit_av(bi, hl, ctx_tiles, pps, uc)
                    for _ in range(pops[gi]):
                        if queue:
                            queue.pop(0)()
                # drain the last AV group c2-major: chunk 0's accumulation
                # stops early, so its norm chain (rcp -> broadcast -> mul ->
                # DMA) overlaps chunk 1's AV streams.
                for c2 in range(2):
                    for uc in groups[-1]:
                        nc.tensor.matmul(
                            ctx_tiles[c2][:],
                            v_sb[:, bi * 8 + uc, hl, :],
                            pps[uc][:, c2 * CH:(c2 + 1) * CH],
                            start=False, stop=(uc == nt - 1))
                    if queue:
                        queue.pop(0)()
                emit_norm(bi, hl, ctx_tiles)

    nc.compile()
    return nc


def _get_nc(NT):
    nc = _compiled.get(NT)
    if nc is None:
        nc = _compiled[NT] = _build(NT)
    return nc


def prepare_in_maps(hidden_states, attention_mask, Wq, bq, Wk, bk, Wv, bv):
    """Returns (in_maps, perms, NT).

    Tokens of each batch are permuted so masked key positions (additive
    mask <= -5000 => exp underflows to exactly 0) come last; the kernel
    then only computes scores/AV for the first NT[bi] 128-key tiles.
    Queries are permuted identically (the query axis is pointwise through
    the whole attention chain) and un-permuted on the host when gathering
    the output, so no second hidden-states copy is needed.
    """
    bf16 = ml_dtypes.bfloat16

    hs = np.asarray(hidden_states, dtype=np.float32)            # [S, B, D]
    mask2 = np.asarray(attention_mask, dtype=np.float32).reshape(B, S)
    perms, NT = [], []
    hsp = np.empty_like(hs)
    maskp = np.empty_like(mask2)
    for bi in range(B):
        masked = mask2[bi] <= -5000.0
        order = np.argsort(masked, kind="stable")   # unmasked keys first
        nk = int((~masked).sum())
        NT.append(min(DCH, max(1, -(-nk // P))))
        perms.append(order)
        hsp[:, bi, :] = hs[order, bi, :]
        maskp[bi] = mask2[bi, order]
    NT = tuple(NT)

    hT = np.ascontiguousarray(hsp.transpose(2, 1, 0).reshape(D, BS)).astype(bf16)
    maskT = np.ascontiguousarray(maskp.T)                       # [S, B]
    Wq = np.asarray(Wq, dtype=np.float32)
    Wk = np.asarray(Wk, dtype=np.float32)
    Wv = np.asarray(Wv, dtype=np.float32)
    bq = np.asarray(bq, dtype=np.float32)
    bk = np.asarray(bk, dtype=np.float32)
    bv = np.asarray(bv, dtype=np.float32)

    # maskT packed as [p, uc, b] -> [128, 32]
    mask_pk = maskT.reshape(DCH, P, B).transpose(1, 0, 2).reshape(P, DCH * B)
    in_maps = []
    for c in range(NCORES):
        sl = slice(P * c, P * (c + 1))
        misc = np.empty((P, 2 + P + DCH * B), dtype=np.float32)
        misc[:, 0] = bq[sl]
        misc[:, 1] = bk[sl]
        misc[:, 2:2 + P] = np.broadcast_to(bv[sl][None, :], (P, P))
        misc[:, 2 + P:] = mask_pk
        def pack_w(W):
            # [D, 128] -> [p, dc*m]: per-partition 2KB contiguous DMA lines
            wt = np.ascontiguousarray(W[sl, :].T).astype(bf16)
            return np.ascontiguousarray(
                wt.reshape(DCH, P, P).transpose(1, 0, 2).reshape(P, DCH * P))

        in_maps.append({
            "hT": hT,
            "wqT": pack_w(Wq),
            "wkT": pack_w(Wk),
            "wvT": pack_w(Wv),
            "misc": misc,
        })
    return in_maps, perms, NT


def kernel(hidden_states, attention_mask, Wq, bq, Wk, bk, Wv, bv):
    global last_exec_time_ns, last_results
    from concourse.bass_utils import run_bass_kernel_spmd

    in_maps, perms, NT = prepare_in_maps(hidden_states, attention_mask,
                                         Wq, bq, Wk, bk, Wv, bv)
    nc = _get_nc(NT)

    trace = bool(int(os.environ.get("KERNEL_TRACE", "0")))
    tmpdir = os.environ.get("KERNEL_TRACE_DIR") or None
    res = run_bass_kernel_spmd(nc, in_maps, core_ids=list(range(NCORES)),
                               trace=trace, tmpdir=tmpdir)
    last_exec_time_ns = res.exec_time_ns
    last_results = res

    # gather: per-core out [B, HPC, DH, S] -> full [S, B, D], un-permuting
    # the query axis per batch
    outs = np.stack([np.asarray(res.results[c]["out"]) for c in range(NCORES)],
                    axis=0)                                     # [C, B, HPC, DH, S]
    full_p = outs.transpose(4, 1, 0, 2, 3).reshape(S, B, D)     # t', b, (c, hl, j)
    full = np.empty_like(full_p)
    for bi in range(B):
        full[perms[bi], bi, :] = full_p[:, bi, :]
    return np.ascontiguousarray(full.astype(np.float32))


# revision 29
# speedup vs baseline: 1.0237x; 1.0123x over previous
"""BertSelfAttention Trainium2 kernel.

Shapes: hidden_states [S=1024, B=4, D=1024], H=16 heads of DH=64.
Sharding: 2 heads per core (8 cores). Each core receives the full hidden
states (pre-transposed + bf16-cast on host) and a 128-row slice of each
projection weight, computes the full attention chain for its two heads with
no cross-core communication, and writes ctx^T per (batch, head).

Device-side layout tricks:
  - masked-key packing: tokens of each batch are permuted on the host so
    masked key positions (additive mask -10000 => exp underflows to
    exactly 0.0) come last; the kernel only computes scores/exp/AV for
    the first NT[bi] (typically 7 of 8) 128-key tiles.  Queries are
    permuted identically (the query axis is pointwise through the chain)
    and un-permuted in the host-side output gather, so the SAME hidden
    buffer feeds Q, K and V and the result is bit-identical.
  - scores are computed transposed (scoresT[u, t] = q_t . k_u) so the
    additive attention mask (per key position u) is a per-partition bias
    that fuses into the Exp activation: probsT = exp(scores/8 + mask).
  - V carries a prepended ones-column, so the AV matmul produces the
    softmax denominator in row 0 of ctxT for free; normalization is
    reciprocal_approx_fast + gpsimd partition_broadcast + one multiply.
  - work is organized as 8 passes, one per (batch, head): scores/exp
    stream per pair of 128-key tiles (pairing halves the sem-guarded PE
    stationary switches), and the AV accumulation chases the exp output
    one pair behind WITHIN the pass, so there is no epilogue AV block.
    V and Q/K projections are woven into the passes as PE filler at
    least one pass ahead of their consumer, in DMA arrival order.
  - weights are pre-tiled on the host so every input DMA moves 2KB
    contiguous per-partition lines; batch 0's hidden pieces split across
    both HWDGE queues; dense single-tile warmup matmuls fill the HAM
    activity window so the 2.4GHz boost arrives during the prologue.

Measured on trn2 (8 cores): see test.py; rel err ~4.4e-3 vs the fp32
reference (bf16 matmul inputs; fp32 accumulation throughout).
"""

import os
import numpy as np
import ml_dtypes

S, B, D, H = 1024, 4, 1024, 16
DH = D // H          # 64
NCORES = 8
HPC = H // NCORES    # heads per core = 2
P = 128              # partitions / d-tile / u-tile
DCH = D // P         # 8 contraction tiles
BS = B * S           # 4096 flattened (b, s)
CH = 512             # matmul free-dim chunk (fp32 psum bank limit)
LAG = 2              # AV runs this many u-tiles behind scores/exp

_compiled = {}
last_exec_time_ns = None
last_results = None


def _build(NT):
    import concourse.bacc as bacc
    import concourse.mybir as mybir
    import concourse.tile as tile
    from contextlib import ExitStack

    f32 = mybir.dt.float32
    bf16 = mybir.dt.bfloat16
    AF = mybir.ActivationFunctionType

    nc = bacc.Bacc("TRN2", target_bir_lowering=False, debug=False,
                   num_devices=NCORES)

    hT_d = nc.dram_tensor("hT", [D, BS], bf16, kind="ExternalInput")
    # weights pre-tiled on host to [p, dc*m] so the DMA moves one
    # contiguous 2KB line per partition (256B strided lines run ~6x slower)
    wqT_d = nc.dram_tensor("wqT", [P, DCH * P], bf16, kind="ExternalInput")
    wkT_d = nc.dram_tensor("wkT", [P, DCH * P], bf16, kind="ExternalInput")
    wvT_d = nc.dram_tensor("wvT", [P, DCH * P], bf16, kind="ExternalInput")
    # packed per-partition constants: [bq | bk | bvb(128) | maskT(8*4)]
    misc_d = nc.dram_tensor("misc", [P, 2 + P + DCH * B], f32,
                            kind="ExternalInput")
    out_d = nc.dram_tensor("out", [B, HPC, DH, S], f32, kind="ExternalOutput")

    with tile.TileContext(nc) as tc, ExitStack() as ctx:
        persist = ctx.enter_context(tc.tile_pool(name="persist", bufs=1))
        probs_pool = ctx.enter_context(tc.tile_pool(name="probs", bufs=8))
        small = ctx.enter_context(tc.tile_pool(name="small", bufs=4))
        out_pool = ctx.enter_context(tc.tile_pool(name="outp", bufs=4))
        ps_mm = ctx.enter_context(tc.tile_pool(name="ps_mm", bufs=2, space="PSUM"))
        ps_sc = ctx.enter_context(tc.tile_pool(name="ps_sc", bufs=2, space="PSUM"))
        ps_ctx = ctx.enter_context(tc.tile_pool(name="ps_ctx", bufs=2, space="PSUM"))

        # ---- persistent SBUF tensors ----
        hT_sb = persist.tile([P, DCH, BS], bf16)        # hidden^T, d-tiled
        wq_sb = persist.tile([P, DCH, P], bf16)
        wk_sb = persist.tile([P, DCH, P], bf16)
        wv_sb = persist.tile([P, DCH, P], bf16)
        misc_sb = persist.tile([P, 2 + P + DCH * B], f32)
        qT_sb = persist.tile([P, BS], bf16)             # Q^T [i, t]
        kT_sb = persist.tile([P, BS], bf16)             # K^T [i, t]
        # V in [t, j] layout + ones column per head: [t-part, t-tile, head, DH+1]
        v_sb = persist.tile([P, BS // P, HPC, DH + 1], bf16)
        dummy_sb = persist.tile([P, CH], bf16)
        ones_sb = persist.tile([1, DH + 1], f32)    # for PE-side broadcast

        bq_sb = misc_sb[:, 0:1]
        bk_sb = misc_sb[:, 1:2]
        bvb_sb = misc_sb[:, 2:2 + P]

        def mask_bias(uc, bi):
            c = 2 + P + uc * B + bi
            return misc_sb[:, c:c + 1]

        # ---- HAM warmup: dead matmuls keep the PE busy while the first
        # weight/hidden DMAs land.  All into ONE psum tile (pure in-order
        # WAW on the PE, no cross-engine sems) so they run back-to-back and
        # fill the HAM activity window — the clock boosts ~3.4us in instead
        # of ~7us into the real work.
        nc.vector.memset(dummy_sb[:], 0.0)
        # prefetch the ACT exp table (~2.7us ACT_TABLE_LOAD) long before the
        # first real exp, so it never blocks the scores->exp->AV chain
        warm_act = small.tile([1, 8], f32, name="warm_act")
        nc.scalar.activation(warm_act[:], dummy_sb[0:1, 0:8], AF.Exp)
        # ~16 back-to-back dummies span the ~7us DMA-latency window before
        # the first hidden pieces land (~0.6us each at the cold 1.2GHz,
        # ~0.2us once boosted); they cost nothing the PE could otherwise do.
        d_ps = ps_sc.tile([P, CH], f32, tag="sc", name="d_ps")
        for _ in range(16):
            nc.tensor.matmul(d_ps[:], dummy_sb[:, 0:P], dummy_sb[:],
                             start=True, stop=True)

        # ---- input DMAs ----
        # Ordered for time-to-first-score-matmul: wq/wk, then batch 0's hT
        # pieces (dc-minor) spread over FOUR HWDGE queues (SP/ACT/DVE/POOL)
        # so the ~2.3MB the prologue needs lands in ~3us instead of ~12,
        # then wv/misc, then the remaining batches on the idle-ish queues.
        hT_re = hT_d.ap().rearrange("(dc p) t -> p dc t", p=P)

        def hT_piece(q, dc, eng):
            qsl = slice(q * S, (q + 1) * S)
            eng.dma_start(hT_sb[:, dc, qsl], hT_re[:, dc, qsl])

        nc.sync.dma_start(wq_sb[:], wqT_d.ap().rearrange("p (dc m) -> p dc m", m=P))
        nc.scalar.dma_start(wk_sb[:], wkT_d.ap().rearrange("p (dc m) -> p dc m", m=P))
        for dc in range(DCH):
            hT_piece(0, dc, nc.sync if dc % 2 == 0 else nc.scalar)
        nc.sync.dma_start(misc_sb[:], misc_d.ap())
        nc.sync.dma_start(wv_sb[:], wvT_d.ap().rearrange("p (dc m) -> p dc m", m=P))
        for q in range(1, B):
            for dc in range(DCH):
                hT_piece(q, dc, nc.sync)

        nc.vector.memset(v_sb[:, :, :, 0:1], 1.0)
        nc.vector.memset(ones_sb[:], 1.0)

        scale = 1.0 / float(np.sqrt(DH))

        # ---- projection thunks (PE filler woven into the passes) ----
        # Q covers all S queries per batch; K/V only the first NT[bi]*128
        # packed (unmasked-first) key positions.
        def emit_qk_chunk(w_sb, b_sb, dst, bi, ci, width):
            sl = slice(bi * S + ci * CH, bi * S + ci * CH + width)
            qk_ps = ps_mm.tile([P, CH], f32, tag="mm", name="qk_ps")
            for dc in range(DCH):
                nc.tensor.matmul(
                    qk_ps[:, 0:width], w_sb[:, dc, :], hT_sb[:, dc, sl],
                    start=(dc == 0), stop=(dc == DCH - 1))
            nc.vector.tensor_scalar_add(dst[:, sl], qk_ps[:, 0:width], b_sb[:])

        def emit_v_tile(tt):
            tsl = slice(tt * P, (tt + 1) * P)
            v_ps = ps_mm.tile([P, P], f32, tag="mm", name="v_ps")
            for dc in range(DCH):
                nc.tensor.matmul(
                    v_ps[:], hT_sb[:, dc, tsl], wv_sb[:, dc, :],
                    start=(dc == 0), stop=(dc == DCH - 1))
            nc.vector.tensor_add(
                v_sb[:, tt, 0:HPC, 1:DH + 1],
                v_ps[:].rearrange("p (h j) -> p h j", j=DH),
                bvb_sb[:].rearrange("p (h j) -> p h j", j=DH))

        def k_widths(bi):
            kcols = NT[bi] * P
            ws = []
            c = 0
            while kcols > 0:
                ws.append((c, min(CH, kcols)))
                kcols -= CH
                c += 1
            return ws

        def qk_thunks(bi):
            th = [lambda c=ci: emit_qk_chunk(wq_sb, bq_sb, qT_sb, bi, c, CH)
                  for ci in range(2)]
            th += [lambda c=ci, w=wd: emit_qk_chunk(wk_sb, bk_sb, kT_sb, bi, c, w)
                   for ci, wd in k_widths(bi)]
            return th

        def v_thunks(bi):
            return [lambda t=tt: emit_v_tile(t)
                    for tt in range(8 * bi, 8 * bi + NT[bi])]

        # ---- prologue: batch 0's Q (both chunks) + K chunk 0, dc-major so
        # all three PSUM accumulation groups chase the arriving hT pieces
        # concurrently.  K chunk 1 is the first filler of pass (0,h0): the
        # first 4 u-tiles of scores only need K chunk 0, so scores start
        # ~1.7us sooner.
        kw0 = min(CH, NT[0] * P)
        pro_pools = (ps_mm, ps_mm, ps_sc)
        pro_specs = [(wq_sb, bq_sb, qT_sb, 0, CH), (wq_sb, bq_sb, qT_sb, 1, CH),
                     (wk_sb, bk_sb, kT_sb, 0, kw0)]
        pro_ps = [pool.tile([P, CH], f32, tag="mm" if pool is ps_mm else "sc",
                            name="pro_ps") for pool in pro_pools]
        for dc in range(DCH):
            for g, (w_sb, b_sb, dst, ci, wd) in enumerate(pro_specs):
                nc.tensor.matmul(
                    pro_ps[g][:, 0:wd], w_sb[:, dc, :],
                    hT_sb[:, dc, ci * CH:ci * CH + wd],
                    start=(dc == 0), stop=(dc == DCH - 1))
        for g, (w_sb, b_sb, dst, ci, wd) in enumerate(pro_specs):
            osl = slice(ci * CH, ci * CH + wd)
            if g < 2:      # split the bias-adds across DVE and ACT
                nc.vector.tensor_scalar_add(dst[:, osl], pro_ps[g][:, 0:wd],
                                            b_sb[:])
            else:
                nc.scalar.activation(dst[:, osl], pro_ps[g][:, 0:wd],
                                     AF.Identity, bias=b_sb[:])

        # ---- per-(batch, head) passes ----
        # Per pass: 8 u-tiles of scoresT -> exp (ACT-paced via the 2-slot
        # sc psum rotation), AV chasing LAG u-tiles behind, projection
        # filler popped once per step.  Norm + output DMA at pass end
        # overlap the next pass.
        def emit_av(bi, hl, ctx_tiles, pps, uc):
            # mid-pass AV step; the final u-tile group is drained c2-major
            # at pass end, so stop is never set here
            for c2 in range(2):
                nc.tensor.matmul(
                    ctx_tiles[c2][:],
                    v_sb[:, bi * 8 + uc, hl, :],
                    pps[uc][:, c2 * CH:(c2 + 1) * CH],
                    start=(uc == 0), stop=False)

        def emit_norm(bi, hl, ctx_tiles, pe_bcast=False):
            # ctx row 0 = denominator, rows 1..DH = unnormalized ctx^T.
            # pe_bcast: broadcast 1/denom across partitions with a PE
            # outer-product (ones x rcp) instead of the ~1us gpsimd
            # partition_broadcast — used for the final batch, where the PE
            # is idle and the norm chain is the kernel's critical tail.
            o_sb = out_pool.tile([DH + 1, S], f32, name="o_sb")
            for c2 in range(2):
                csl = slice(c2 * CH, (c2 + 1) * CH)
                ctx_ps = ctx_tiles[c2]
                rcp_sb = small.tile([1, CH], f32, name="rcp_sb")
                nc.vector.reciprocal_approx_fast(rcp_sb[:], ctx_ps[0:1, :])
                if pe_bcast:
                    rcpb_ps = ps_mm.tile([DH + 1, CH], f32, tag="mm",
                                         name="rcpb_ps")
                    nc.tensor.matmul(rcpb_ps[:], ones_sb[:], rcp_sb[:],
                                     start=True, stop=True)
                    nc.vector.tensor_mul(o_sb[:, csl], ctx_ps[:], rcpb_ps[:])
                else:
                    rcpb_sb = small.tile([DH + 1, CH], f32, name="rcpb_sb")
                    nc.gpsimd.partition_broadcast(rcpb_sb[:], rcp_sb[:])
                    nc.vector.tensor_mul(o_sb[:, csl], ctx_ps[:], rcpb_sb[:])
                nc.sync.dma_start(out_d.ap()[bi, hl, :, csl],
                                  o_sb[1:DH + 1, csl])

        # filler queues per pass.  Each projection finishes at least one
        # full pass before its consumer pass, so the consumer's matmuls
        # carry no cross-engine waits (the PE's vector clock has already
        # observed the producing DVE/ACT ticks).  v(b0) is the exception:
        # it chases pass (0,h0)'s own AV, one u-tile pair ahead — hence the
        # front-loaded pop distribution below.
        k1_thunk = [lambda c=ci, w=wd: emit_qk_chunk(wk_sb, bk_sb, kT_sb,
                                                     0, c, w)
                    for ci, wd in k_widths(0)[1:]]
        qk1 = qk_thunks(1)
        fillers = {
            (0, 0): k1_thunk + v_thunks(0) + qk1[:2],
            (0, 1): qk1[2:] + v_thunks(1),
            (1, 0): qk_thunks(2),
            (1, 1): v_thunks(2),
            (2, 0): qk_thunks(3),
            (2, 1): v_thunks(3),
            (3, 0): [], (3, 1): [],
        }

        def pop_split(n, g):
            base, extra = divmod(n, g)
            return [base + (1 if i < extra else 0) for i in range(g)]

        # u-tiles processed in pairs: each group emits scores+exp for two
        # u-tiles back-to-back (both sc psum slots in flight), then the AV
        # pair one group behind, then the group's filler thunks.  Coarser
        # grouping halves the number of sem-guarded PE stationary switches.
        for bi in range(B):
            nt = NT[bi]
            groups = [tuple(range(g, min(g + 2, nt))) for g in range(0, nt, 2)]
            for hl in range(HPC):
                queue = fillers[(bi, hl)]
                pops = pop_split(len(queue), len(groups))
                hsl = slice(hl * DH, (hl + 1) * DH)
                ctx_tiles = [ps_ctx.tile([DH + 1, CH], f32, tag="ctx",
                                         name="ctx_ps") for _ in range(2)]
                pps = []
                for gi, grp in enumerate(groups):
                    for uc in grp:
                        usl = slice(bi * S + uc * P, bi * S + (uc + 1) * P)
                        sc = ps_sc.tile([P, S], f32, tag="sc", name="sc_ps")
                        for c2 in range(2):
                            qsl = slice(bi * S + c2 * CH,
                                        bi * S + (c2 + 1) * CH)
                            nc.tensor.matmul(
                                sc[:, c2 * CH:(c2 + 1) * CH],
                                kT_sb[hsl, usl], qT_sb[hsl, qsl],
                                start=True, stop=True)
                        pp = probs_pool.tile([P, S], bf16, name="pp")
                        nc.scalar.activation(pp[:], sc[:], AF.Exp,
                                             bias=mask_bias(uc, bi),
                                             scale=scale)
                        pps.append(pp)
                    if gi >= 1:
                        for uc in groups[gi - 1]:
                            emit_av(bi, hl, ctx_tiles, pps, uc)
                    for _ in range(pops[gi]):
                        if queue:
                            queue.pop(0)()
                # drain the last AV group c2-major: chunk 0's accumulation
                # stops early, so its norm chain (rcp -> broadcast -> mul ->
                # DMA) overlaps chunk 1's AV streams.
                for c2 in range(2):
                    for uc in groups[-1]:
                        nc.tensor.matmul(
                            ctx_tiles[c2][:],
                            v_sb[:, bi * 8 + uc, hl, :],
                            pps[uc][:, c2 * CH:(c2 + 1) * CH],
                            start=False, stop=(uc == nt - 1))
                    if queue:
                        queue.pop(0)()
                emit_norm(bi, hl, ctx_tiles)

    nc.compile()
    return nc


def _get_nc(NT):
    nc = _compiled.get(NT)
    if nc is None:
        nc = _compiled[NT] = _build(NT)
    return nc


def prepare_in_maps(hidden_states, attention_mask, Wq, bq, Wk, bk, Wv, bv):
    """Returns (in_maps, perms, NT).

    Tokens of each batch are permuted so masked key positions (additive
    mask <= -5000 => exp underflows to exactly 0) come last; the kernel
    then only computes scores/AV for the first NT[bi] 128-key tiles.
    Queries are permuted identically (the query axis is pointwise through
    the whole attention chain) and un-permuted on the host when gathering
    the output, so no second hidden-states copy is needed.
    """
    bf16 = ml_dtypes.bfloat16

    hs = np.asarray(hidden_states, dtype=np.float32)            # [S, B, D]
    mask2 = np.asarray(attention_mask, dtype=np.float32).reshape(B, S)
    perms, NT = [], []
    hsp = np.empty_like(hs)
    maskp = np.empty_like(mask2)
    for bi in range(B):
        masked = mask2[bi] <= -5000.0
        order = np.argsort(masked, kind="stable")   # unmasked keys first
        nk = int((~masked).sum())
        NT.append(min(DCH, max(1, -(-nk // P))))
        perms.append(order)
        hsp[:, bi, :] = hs[order, bi, :]
        maskp[bi] = mask2[bi, order]
    NT = tuple(NT)

    hT = np.ascontiguousarray(hsp.transpose(2, 1, 0).reshape(D, BS)).astype(bf16)
    maskT = np.ascontiguousarray(maskp.T)                       # [S, B]
    Wq = np.asarray(Wq, dtype=np.float32)
    Wk = np.asarray(Wk, dtype=np.float32)
    Wv = np.asarray(Wv, dtype=np.float32)
    bq = np.asarray(bq, dtype=np.float32)
    bk = np.asarray(bk, dtype=np.float32)
    bv = np.asarray(bv, dtype=np.float32)

    # maskT packed as [p, uc, b] -> [128, 32]
    mask_pk = maskT.reshape(DCH, P, B).transpose(1, 0, 2).reshape(P, DCH * B)
    in_maps = []
    for c in range(NCORES):
        sl = slice(P * c, P * (c + 1))
        misc = np.empty((P, 2 + P + DCH * B), dtype=np.float32)
        misc[:, 0] = bq[sl]
        misc[:, 1] = bk[sl]
        misc[:, 2:2 + P] = np.broadcast_to(bv[sl][None, :], (P, P))
        misc[:, 2 + P:] = mask_pk
        def pack_w(W):
            # [D, 128] -> [p, dc*m]: per-partition 2KB contiguous DMA lines
            wt = np.ascontiguousarray(W[sl, :].T).astype(bf16)
            return np.ascontiguousarray(
                wt.reshape(DCH, P, P).transpose(1, 0, 2).reshape(P, DCH * P))

        in_maps.append({
            "hT": hT,
            "wqT": pack_w(Wq),
            "wkT": pack_w(Wk),
            "wvT": pack_w(Wv),
            "misc": misc,
        })
    return in_maps, perms, NT


def kernel(hidden_states, attention_mask, Wq, bq, Wk, bk, Wv, bv):
    global last_exec_time_ns, last_results
    from concourse.bass_utils import run_bass_kernel_spmd

    in_maps, perms, NT = prepare_in_maps(hidden_states, attention_mask,
                                         Wq, bq, Wk, bk, Wv, bv)
    nc = _get_nc(NT)

    trace = bool(int(os.environ.get("KERNEL_TRACE", "0")))
    tmpdir = os.environ.get("KERNEL_TRACE_DIR") or None
    res = run_bass_kernel_spmd(nc, in_maps, core_ids=list(range(NCORES)),
                               trace=trace, tmpdir=tmpdir)
    last_exec_time_ns = res.exec_time_ns
    last_results = res

    # gather: per-core out [B, HPC, DH, S] -> full [S, B, D], un-permuting
    # the query axis per batch
    outs = np.stack([np.asarray(res.results[c]["out"]) for c in range(NCORES)],
                    axis=0)                                     # [C, B, HPC, DH, S]
    full_p = outs.transpose(4, 1, 0, 2, 3).reshape(S, B, D)     # t', b, (c, hl, j)
    full = np.empty_like(full_p)
    for bi in range(B):
        full[perms[bi], bi, :] = full_p[:, bi, :]
    return np.ascontiguousarray(full.astype(np.float32))
